# revision 4
# baseline (speedup 1.0000x reference)
"""DenseDilatedKnnGraph edge-extraction kernel for Trainium2 (8 NeuronCores).

Strategy (per the batch-sharding hint, refined to 2 cores per batch element):
  - core c handles batch b = c//2, row half h = c%2 (2048 of 4096 rows).
  - Host: L2-normalize x over the channel dim (mirrors F.normalize), compute
    the per-batch standardization statistics analytically in fp64 from the
    tiny C x C Gram matrix (sum_ij G_ij = |X.1|^2, sum_ij G_ij^2 = |X X^T|_F^2),
    and fold them into a single per-batch threshold t_b on the N x N Gram
    matrix G:   adjacency  <=>  G_ij > t_b.
  - Device: G row-blocks via TensorE fp16 hi/lo split (3 passes: hi.hi +
    hi.lo + lo.hi, inputs pre-scaled by 256 so fp16 subnormals never occur)
    which measures at fp32 accuracy while running at ~3x the speed of native
    fp32 matmul. Edges come out of two fused vector ops per tile:
        e1i = (G > t) * (b*N + j + 1)       [scalar_tensor_tensor, DVE]
        e0i = (e1i > 0) * (b*N + i + 1)     [tensor_scalar, DVE 2x mode]
    then ScalarE casts with a fused -1 bias to int32, and DMA streams the
    two [2048, 4096] int32 edge planes out.
  - Host: concatenate the 8 contiguous row spans -> (2, B*N*N) int32.
"""

import sys

for _p in ("/opt/trn_rl_repo", "/root/.axon_site/_ro/trn_rl_repo"):
    if _p not in sys.path:
        sys.path.append(_p)

import numpy as np

B, C, N = 4, 384, 4096
HALF = N // 2          # rows per core
KT = C // 128          # k tiles (3)
NCORES = 2 * B
PPF_09 = 1.2815515655446004
EPS = 1e-12
SCALE = 256.0          # host pre-scale; G comes out scaled by SCALE**2
RB = HALF // 128       # row blocks per core (16)
JH = 2                 # j-halves per row block (2048 cols each)
JT = 4                 # 512-col matmul tiles per j-half
XCOLS = HALF + N       # stationary cols then moving cols, per k tile

_compiled_nc = None


def _build_nc():
    import concourse.bacc as bacc
    import concourse.tile as tile
    import concourse.mybir as mybir

    f32 = mybir.dt.float32
    f16 = mybir.dt.float16
    i32 = mybir.dt.int32
    Alu = mybir.AluOpType
    Act = mybir.ActivationFunctionType

    nc = bacc.Bacc("TRN2", target_bir_lowering=False, debug=False)

    xhi_d = nc.dram_tensor("xhi", [KT, 128, XCOLS], f16, kind="ExternalInput")
    xlo_d = nc.dram_tensor("xlo", [KT, 128, XCOLS], f16, kind="ExternalInput")
    thr_d = nc.dram_tensor("thr", [128, 1], f32, kind="ExternalInput")
    row_d = nc.dram_tensor("rowp1", [128, RB], f32, kind="ExternalInput")
    col_d = nc.dram_tensor("colp1", [128, N], f32, kind="ExternalInput")
    e0_d = nc.dram_tensor("e0", [HALF, N], i32, kind="ExternalOutput")
    e1_d = nc.dram_tensor("e1", [HALF, N], i32, kind="ExternalOutput")

    with tile.TileContext(nc) as tc:
        with tc.tile_pool(name="const", bufs=1) as cpool, \
             tc.tile_pool(name="psum", bufs=6, space="PSUM") as psum, \
             tc.tile_pool(name="e1ip", bufs=3) as e1ip, \
             tc.tile_pool(name="e0ip", bufs=2) as e0ip, \
             tc.tile_pool(name="outp", bufs=4) as outp:
            his = [cpool.tile([128, XCOLS], f16, name=f"hi{k}") for k in range(KT)]
            los = [cpool.tile([128, XCOLS], f16, name=f"lo{k}") for k in range(KT)]
            # hi tiles first (the hi.hi passes run while lo still streams),
            # alternating the two HWDGE queues for bandwidth.
            qs = [nc.sync, nc.scalar]
            for k in range(KT):
                qs[k % 2].dma_start(out=his[k][:], in_=xhi_d[k])
            for k in range(KT):
                qs[(k + 1) % 2].dma_start(out=los[k][:], in_=xlo_d[k])
            thr_t = cpool.tile([128, 1], f32, name="thr_t")
            nc.gpsimd.dma_start(out=thr_t[:], in_=thr_d.ap())
            row_t = cpool.tile([128, RB], f32, name="row_t")
            nc.gpsimd.dma_start(out=row_t[:], in_=row_d.ap())
            col_t = cpool.tile([128, N], f32, name="col_t")
            nc.gpsimd.dma_start(out=col_t[:], in_=col_d.ap())

            for rb in range(RB):
                i0 = rb * 128
                for jh in range(JH):
                    e1i = e1ip.tile([128, N // JH], f32, name="e1i")
                    for jt in range(JT):
                        j0 = jh * (N // JH) + jt * 512
                        ps = psum.tile([128, 512], f32, name="ps")
                        pairs = [(his[k], his[k]) for k in range(KT)]
                        for k in range(KT):
                            pairs += [(his[k], los[k]), (los[k], his[k])]
                        for m, (a, b) in enumerate(pairs):
                            nc.tensor.matmul(
                                ps[:],
                                a[:, i0:i0 + 128],
                                b[:, HALF + j0:HALF + j0 + 512],
                                start=(m == 0), stop=(m == 3 * KT - 1),
                            )
                        # e1i = (G > t) * (b*N + j + 1)
                        nc.vector.scalar_tensor_tensor(
                            e1i[:, jt * 512:(jt + 1) * 512], ps[:], thr_t[:],
                            col_t[:, j0:j0 + 512], op0=Alu.is_gt, op1=Alu.mult,
                        )
                    # e0i = (e1i > 0) * (b*N + i + 1)
                    e0i = e0ip.tile([128, N // JH], f32, name="e0i")
                    nc.vector.tensor_scalar(
                        e0i[:], e1i[:], 0.0, row_t[:, rb:rb + 1],
                        op0=Alu.is_gt, op1=Alu.mult,
                    )
                    e0o = outp.tile([128, N // JH], i32, name="e0o")
                    e1o = outp.tile([128, N // JH], i32, name="e1o")
                    nc.scalar.activation(e0o[:], e0i[:], Act.Copy, bias=-1.0)
                    nc.scalar.activation(e1o[:], e1i[:], Act.Copy, bias=-1.0)
                    jcol = jh * (N // JH)
                    nc.sync.dma_start(
                        out=e0_d.ap()[i0:i0 + 128, jcol:jcol + N // JH], in_=e0o[:])
                    nc.scalar.dma_start(
                        out=e1_d.ap()[i0:i0 + 128, jcol:jcol + N // JH], in_=e1o[:])
    nc.compile()
    return nc


def get_nc():
    global _compiled_nc
    if _compiled_nc is None:
        _compiled_nc = _build_nc()
    return _compiled_nc


def make_inputs(x):
    """Host-side prep: normalize, stats -> thresholds, per-core input maps."""
    xs = np.asarray(x)[:, :, :, 0]                      # (B, C, N) fp32
    nrm = np.sqrt(np.sum(xs * xs, axis=1, keepdims=True))
    xn = xs / np.maximum(nrm, EPS)                      # fp32, mirrors reference

    Nsq = float(N) * float(N)
    in_maps = []
    for b in range(B):
        xb64 = xn[b].astype(np.float64)                 # (C, N)
        s = xb64.sum(axis=1)
        M = xb64 @ xb64.T                               # (C, C)
        sum_g = float(s @ s)
        sum_g2 = float((M * M).sum())
        mean = (2.0 * sum_g - 2.0 * Nsq) / Nsq
        s2 = 4.0 * sum_g2 - 8.0 * sum_g + 4.0 * Nsq
        var = (s2 - Nsq * mean * mean) / (Nsq - 1.0)
        t_b = (mean + PPF_09 * np.sqrt(var) + 2.0) / 2.0
        thr_dev = np.full((128, 1), t_b * SCALE * SCALE, np.float32)

        xbs = (xn[b] * SCALE).astype(np.float32)        # (C, N), scaled
        colp1 = np.ascontiguousarray(np.broadcast_to(
            (b * N + np.arange(N) + 1).astype(np.float32), (128, N)))
        for h in range(2):
            stat = xbs[:, h * HALF:(h + 1) * HALF]
            xcat = np.concatenate([stat, xbs], axis=1)  # (C, XCOLS)
            hi = xcat.astype(np.float16)
            lo = (xcat - hi.astype(np.float32)).astype(np.float16)
            rows = (b * N + h * HALF
                    + (np.arange(RB)[None, :] * 128 + np.arange(128)[:, None])
                    + 1).astype(np.float32)             # (128, RB)
            in_maps.append({
                "xhi": np.ascontiguousarray(hi.reshape(KT, 128, XCOLS)),
                "xlo": np.ascontiguousarray(lo.reshape(KT, 128, XCOLS)),
                "thr": thr_dev,
                "rowp1": np.ascontiguousarray(rows),
                "colp1": colp1,
            })
    return in_maps


def assemble(results):
    out = np.empty((2, B * N * N), np.int32)
    for c in range(NCORES):
        b, h = divmod(c, 2)
        base = b * N * N + h * HALF * N
        out[0, base:base + HALF * N] = results[c]["e0"].ravel()
        out[1, base:base + HALF * N] = results[c]["e1"].ravel()
    return out


def kernel(x):
    from concourse.bass_utils import run_bass_kernel_spmd

    nc = get_nc()
    in_maps = make_inputs(x)
    res = run_bass_kernel_spmd(nc, in_maps, list(range(NCORES)))
    return assemble(res.results)


# revision 7
# speedup vs baseline: 1.2132x; 1.2132x over previous
"""DenseDilatedKnnGraph edge-extraction kernel for Trainium2 (8 NeuronCores).

Strategy (per the batch-sharding hint, refined to 2 cores per batch element):
  - core c handles batch b = c//2, row half h = c%2 (2048 of 4096 rows).
  - Host: L2-normalize x over the channel dim (mirrors F.normalize), compute
    the per-batch standardization statistics analytically in fp64 from the
    tiny C x C Gram matrix (sum_ij G_ij = |X.1|^2, sum_ij G_ij^2 = |X X^T|_F^2),
    and fold them into a single per-batch threshold t_b on the N x N Gram
    matrix G:   adjacency  <=>  G_ij > t_b.
  - Device: G row-blocks via TensorE fp16 hi/lo split (3 passes: hi.hi +
    hi.lo + lo.hi, inputs pre-scaled by 256 so fp16 subnormals never occur)
    which measures at fp32 accuracy while running at ~3x the speed of native
    fp32 matmul. Edges come out of two fused vector ops per tile:
        e1i = (G > t) * (b*N + j + 1)       [scalar_tensor_tensor, DVE]
        e0i = (e1i > 0) * (b*N + i + 1)     [tensor_scalar, DVE 2x mode]
    then ScalarE casts with a fused -1 bias to int32, and DMA streams the
    two [2048, 4096] int32 edge planes out.
  - Host: concatenate the 8 contiguous row spans -> (2, B*N*N) int32.
"""

import sys

for _p in ("/opt/trn_rl_repo", "/root/.axon_site/_ro/trn_rl_repo"):
    if _p not in sys.path:
        sys.path.append(_p)

import numpy as np

B, C, N = 4, 384, 4096
HALF = N // 2          # rows per core
KT = C // 128          # k tiles (3)
NCORES = 2 * B
PPF_09 = 1.2815515655446004
EPS = 1e-12
SCALE = 256.0          # host pre-scale; G comes out scaled by SCALE**2
RB = HALF // 128       # row blocks per core (16)
JH = 2                 # j-halves per row block (2048 cols each)
JT = 4                 # 512-col matmul tiles per j-half
XCOLS = HALF + N       # stationary cols then moving cols, per k tile
HEADC = HALF + 512     # head-start chunk of the k=0 tile

_compiled_nc = None


def _build_nc():
    import concourse.bacc as bacc
    import concourse.tile as tile
    import concourse.mybir as mybir

    f32 = mybir.dt.float32
    f16 = mybir.dt.float16
    i32 = mybir.dt.int32
    Alu = mybir.AluOpType
    Act = mybir.ActivationFunctionType

    nc = bacc.Bacc("TRN2", target_bir_lowering=False, debug=False)

    xh0_d = nc.dram_tensor("xh0", [128, 2, HEADC], f16, kind="ExternalInput")
    x_d = nc.dram_tensor("xhl", [KT, 128, 2, XCOLS], f16, kind="ExternalInput")
    thr_d = nc.dram_tensor("thr", [128, 1], f32, kind="ExternalInput")
    row_d = nc.dram_tensor("rowp1", [128, RB], f32, kind="ExternalInput")
    col_d = nc.dram_tensor("colp1", [128, N], f32, kind="ExternalInput")
    e0_d = nc.dram_tensor("e0", [HALF, N], i32, kind="ExternalOutput")
    e1_d = nc.dram_tensor("e1", [HALF, N], i32, kind="ExternalOutput")

    with tile.TileContext(nc) as tc:
        with tc.tile_pool(name="const", bufs=1) as cpool, \
             tc.tile_pool(name="psum", bufs=6, space="PSUM") as psum, \
             tc.tile_pool(name="e1ip", bufs=3) as e1ip, \
             tc.tile_pool(name="e1if", bufs=2) as e1if, \
             tc.tile_pool(name="e0ip", bufs=2) as e0ip, \
             tc.tile_pool(name="e0if", bufs=2) as e0if, \
             tc.tile_pool(name="outp", bufs=3) as outp, \
             tc.tile_pool(name="outf", bufs=2) as outf:
            # head-start tile: lets the very first matmul group begin after a
            # 1.3 MB DMA instead of the full 3.1 MB k=0 tile.
            xh0 = cpool.tile([128, 2, HEADC], f16, name="xh0")
            nc.sync.dma_start(out=xh0[:], in_=xh0_d.ap())
            xts = [cpool.tile([128, 2, XCOLS], f16, name=f"x{k}") for k in range(KT)]
            for k in range(KT):
                nc.sync.dma_start(out=xts[k][:], in_=x_d[k])
            thr_t = cpool.tile([128, 1], f32, name="thr_t")
            nc.sync.dma_start(out=thr_t[:], in_=thr_d.ap())
            row_t = cpool.tile([128, RB], f32, name="row_t")
            nc.sync.dma_start(out=row_t[:], in_=row_d.ap())
            col_t = cpool.tile([128, N], f32, name="col_t")
            nc.sync.dma_start(out=col_t[:], in_=col_d.ap())

            def mm_group(ps, i0, j0, first_tile):
                m = 0
                for k in range(KT):
                    if first_tile and k == 0:
                        hi = xh0[:, 0, :]
                        lo = xh0[:, 1, :]
                        moff = HALF
                    else:
                        hi = xts[k][:, 0, :]
                        lo = xts[k][:, 1, :]
                        moff = HALF
                    for a, bb in ((hi, hi), (hi, lo), (lo, hi)):
                        nc.tensor.matmul(
                            ps[:],
                            a[:, i0:i0 + 128],
                            bb[:, moff + j0:moff + j0 + 512],
                            start=(m == 0), stop=(m == 3 * KT - 1),
                        )
                        m += 1

            def post(e1i_ap, rb, jcol, width, e0i_pool, out_pool):
                e0i = e0i_pool.tile([128, width], f32, name="e0i")
                nc.vector.tensor_scalar(
                    e0i[:], e1i_ap, 0.0, row_t[:, rb:rb + 1],
                    op0=Alu.is_gt, op1=Alu.mult,
                )
                e0o = out_pool.tile([128, width], i32, name="e0o")
                e1o = out_pool.tile([128, width], i32, name="e1o")
                nc.scalar.activation(e0o[:], e0i[:], Act.Copy, bias=-1.0)
                nc.scalar.activation(e1o[:], e1i_ap, Act.Copy, bias=-1.0)
                i0 = rb * 128
                nc.sync.dma_start(
                    out=e0_d.ap()[i0:i0 + 128, jcol:jcol + width], in_=e0o[:])
                nc.sync.dma_start(
                    out=e1_d.ap()[i0:i0 + 128, jcol:jcol + width], in_=e1o[:])

            for rb in range(RB):
                i0 = rb * 128
                for jh in range(JH):
                    last_block = (rb == RB - 1 and jh == JH - 1)
                    if not last_block:
                        e1i = e1ip.tile([128, N // JH], f32, name="e1i")
                        for jt in range(JT):
                            j0 = jh * (N // JH) + jt * 512
                            ps = psum.tile([128, 512], f32, name="ps")
                            mm_group(ps, i0, j0, rb == 0 and jh == 0 and jt == 0)
                            nc.vector.scalar_tensor_tensor(
                                e1i[:, jt * 512:(jt + 1) * 512], ps[:], thr_t[:],
                                col_t[:, j0:j0 + 512], op0=Alu.is_gt, op1=Alu.mult,
                            )
                        post(e1i[:], rb, jh * (N // JH), N // JH, e0ip, outp)
                    else:
                        # drain the tail at 512-col granularity so the last
                        # DVE/ACT/DMA chain after the final matmul is short
                        for jt in range(JT):
                            j0 = jh * (N // JH) + jt * 512
                            ps = psum.tile([128, 512], f32, name="ps")
                            mm_group(ps, i0, j0, False)
                            e1s = e1if.tile([128, 512], f32, name="e1s")
                            nc.vector.scalar_tensor_tensor(
                                e1s[:], ps[:], thr_t[:],
                                col_t[:, j0:j0 + 512], op0=Alu.is_gt, op1=Alu.mult,
                            )
                            post(e1s[:], rb, j0, 512, e0if, outf)
    nc.compile()
    return nc


def get_nc():
    global _compiled_nc
    if _compiled_nc is None:
        _compiled_nc = _build_nc()
    return _compiled_nc


def make_inputs(x):
    """Host-side prep: normalize, stats -> thresholds, per-core input maps."""
    xs = np.asarray(x)[:, :, :, 0]                      # (B, C, N) fp32
    nrm = np.sqrt(np.sum(xs * xs, axis=1, keepdims=True))
    xn = xs / np.maximum(nrm, EPS)                      # fp32, mirrors reference

    Nsq = float(N) * float(N)
    in_maps = []
    for b in range(B):
        xb64 = xn[b].astype(np.float64)                 # (C, N)
        s = xb64.sum(axis=1)
        M = xb64 @ xb64.T                               # (C, C)
        sum_g = float(s @ s)
        sum_g2 = float((M * M).sum())
        mean = (2.0 * sum_g - 2.0 * Nsq) / Nsq
        s2 = 4.0 * sum_g2 - 8.0 * sum_g + 4.0 * Nsq
        var = (s2 - Nsq * mean * mean) / (Nsq - 1.0)
        t_b = (mean + PPF_09 * np.sqrt(var) + 2.0) / 2.0
        thr_dev = np.full((128, 1), t_b * SCALE * SCALE, np.float32)

        xbs = (xn[b] * SCALE).astype(np.float32)        # (C, N), scaled
        colp1 = np.ascontiguousarray(np.broadcast_to(
            (b * N + np.arange(N) + 1).astype(np.float32), (128, N)))
        for h in range(2):
            stat = xbs[:, h * HALF:(h + 1) * HALF]
            xcat = np.concatenate([stat, xbs], axis=1)  # (C, XCOLS)
            hi = xcat.astype(np.float16)
            lo = (xcat - hi.astype(np.float32)).astype(np.float16)
            hi_k = hi.reshape(KT, 128, XCOLS)
            lo_k = lo.reshape(KT, 128, XCOLS)
            xhl = np.stack([hi_k, lo_k], axis=2)        # (KT, 128, 2, XCOLS)
            xh0 = np.ascontiguousarray(xhl[0, :, :, :HEADC])
            rows = (b * N + h * HALF
                    + (np.arange(RB)[None, :] * 128 + np.arange(128)[:, None])
                    + 1).astype(np.float32)             # (128, RB)
            in_maps.append({
                "xh0": xh0,
                "xhl": np.ascontiguousarray(xhl),
                "thr": thr_dev,
                "rowp1": np.ascontiguousarray(rows),
                "colp1": colp1,
            })
    return in_maps


def assemble(results):
    out = np.empty((2, B * N * N), np.int32)
    for c in range(NCORES):
        b, h = divmod(c, 2)
        base = b * N * N + h * HALF * N
        out[0, base:base + HALF * N] = results[c]["e0"].ravel()
        out[1, base:base + HALF * N] = results[c]["e1"].ravel()
    return out


def kernel(x):
    from concourse.bass_utils import run_bass_kernel_spmd

    nc = get_nc()
    in_maps = make_inputs(x)
    res = run_bass_kernel_spmd(nc, in_maps, list(range(NCORES)))
    return assemble(res.results)


# revision 8
# speedup vs baseline: 1.2275x; 1.0118x over previous
"""v3: adds Gram-symmetry exploitation to the v2b kernel.

Per-core local column order is [own 2048 rows | other 2048 rows], so the
diagonal (symmetric) block of G is always local columns [0, 2048) — SPMD-
uniform across cores. Within that block, strictly-lower [128,512] tiles are
not recomputed: their adjacency masks are PE-transposed copies of the
strictly-upper tiles' masks (bf16 0/1, retained in SBUF). That removes
216 of 1152 matmuls per core at the cost of 96 cheap transpose ops.
Host un-permutes the output columns (a half-swap for odd cores) during
assembly at zero extra copy cost.
"""

import sys

for _p in ("/opt/trn_rl_repo", "/root/.axon_site/_ro/trn_rl_repo"):
    if _p not in sys.path:
        sys.path.append(_p)

import numpy as np

B, C, N = 4, 384, 4096
HALF = N // 2
KT = C // 128
NCORES = 2 * B
PPF_09 = 1.2815515655446004
EPS = 1e-12
SCALE = 256.0
RB = HALF // 128       # 16 row blocks per core
JH = 2
JT = 4
HEADC = 512            # head-start chunk (first matmul tile's moving cols)

_compiled_nc = None


def _build_nc():
    import concourse.bacc as bacc
    import concourse.tile as tile
    import concourse.mybir as mybir

    f32 = mybir.dt.float32
    f16 = mybir.dt.float16
    bf16 = mybir.dt.bfloat16
    i32 = mybir.dt.int32
    Alu = mybir.AluOpType
    Act = mybir.ActivationFunctionType

    nc = bacc.Bacc("TRN2", target_bir_lowering=False, debug=False)

    xh0_d = nc.dram_tensor("xh0", [128, 2, HEADC], f16, kind="ExternalInput")
    x_d = nc.dram_tensor("xhl", [KT, 128, 2, N], f16, kind="ExternalInput")
    thr_d = nc.dram_tensor("thr", [128, 1], f32, kind="ExternalInput")
    row_d = nc.dram_tensor("rowp1", [128, RB], f32, kind="ExternalInput")
    col_d = nc.dram_tensor("colp1", [128, N], f32, kind="ExternalInput")
    idn_d = nc.dram_tensor("ident", [128, 128], bf16, kind="ExternalInput")
    e0_d = nc.dram_tensor("e0", [HALF, N], i32, kind="ExternalOutput")
    e1_d = nc.dram_tensor("e1", [HALF, N], i32, kind="ExternalOutput")

    def tclass(rb, jt):
        if rb <= 4 * jt - 1:
            return "upper"
        if rb >= 4 * jt + 4:
            return "lower"
        return "cross"

    with tile.TileContext(nc) as tc:
        with tc.tile_pool(name="const", bufs=1) as cpool, \
             tc.tile_pool(name="psum", bufs=5, space="PSUM") as psum, \
             tc.tile_pool(name="pstp", bufs=2, space="PSUM") as pstp, \
             tc.tile_pool(name="e1ip", bufs=3) as e1ip, \
             tc.tile_pool(name="e1if", bufs=2) as e1if, \
             tc.tile_pool(name="e0ip", bufs=2) as e0ip, \
             tc.tile_pool(name="e0if", bufs=2) as e0if, \
             tc.tile_pool(name="outp", bufs=3) as outp, \
             tc.tile_pool(name="outf", bufs=2) as outf:
            xh0 = cpool.tile([128, 2, HEADC], f16, name="xh0")
            nc.sync.dma_start(out=xh0[:], in_=xh0_d.ap())
            xts = [cpool.tile([128, 2, N], f16, name=f"x{k}") for k in range(KT)]
            for k in range(KT):
                nc.sync.dma_start(out=xts[k][:], in_=x_d[k])
            thr_t = cpool.tile([128, 1], f32, name="thr_t")
            nc.sync.dma_start(out=thr_t[:], in_=thr_d.ap())
            row_t = cpool.tile([128, RB], f32, name="row_t")
            nc.sync.dma_start(out=row_t[:], in_=row_d.ap())
            col_t = cpool.tile([128, N], f32, name="col_t")
            nc.sync.dma_start(out=col_t[:], in_=col_d.ap())
            idn_t = cpool.tile([128, 128], bf16, name="idn_t")
            nc.sync.dma_start(out=idn_t[:], in_=idn_d.ap())
            masks = {}
            for jt in range(JT):
                for rb in range(RB):
                    if tclass(rb, jt) == "upper":
                        masks[(rb, jt)] = cpool.tile(
                            [128, 512], bf16, name=f"mk_{rb}_{jt}")

            def mm_group(ps, i0, j0, first_tile):
                m = 0
                for k in range(KT):
                    src = xh0 if (first_tile and k == 0) else xts[k]
                    hi = src[:, 0, :]
                    lo = src[:, 1, :]
                    for a, bb in ((hi, hi), (hi, lo), (lo, hi)):
                        nc.tensor.matmul(
                            ps[:],
                            a[:, i0:i0 + 128],
                            bb[:, j0:j0 + 512],
                            start=(m == 0), stop=(m == 3 * KT - 1),
                        )
                        m += 1

            def post(e1i_ap, rb, jcol, width, e0i_pool, out_pool):
                e0i = e0i_pool.tile([128, width], f32, name="e0i")
                nc.vector.tensor_scalar(
                    e0i[:], e1i_ap, 0.0, row_t[:, rb:rb + 1],
                    op0=Alu.is_gt, op1=Alu.mult,
                )
                e0o = out_pool.tile([128, width], i32, name="e0o")
                e1o = out_pool.tile([128, width], i32, name="e1o")
                nc.scalar.activation(e0o[:], e0i[:], Act.Copy, bias=-1.0)
                nc.scalar.activation(e1o[:], e1i_ap, Act.Copy, bias=-1.0)
                i0 = rb * 128
                nc.sync.dma_start(
                    out=e0_d.ap()[i0:i0 + 128, jcol:jcol + width], in_=e0o[:])
                nc.sync.dma_start(
                    out=e1_d.ap()[i0:i0 + 128, jcol:jcol + width], in_=e1o[:])

            for rb in range(RB):
                i0 = rb * 128
                # jh=0: diagonal (symmetric) half, local cols [0, 2048)
                e1i = e1ip.tile([128, HALF], f32, name="e1i")
                for jt in range(JT):
                    j0 = jt * 512
                    cls = tclass(rb, jt)
                    sl = e1i[:, j0:j0 + 512]
                    if cls == "lower":
                        pst = pstp.tile([128, 512], bf16, name="pst")
                        for q in range(4):
                            src = masks[(4 * jt + q, rb // 4)]
                            nc.tensor.transpose(
                                pst[:, q * 128:(q + 1) * 128],
                                src[:, (rb % 4) * 128:(rb % 4) * 128 + 128],
                                idn_t[:],
                            )
                        nc.vector.tensor_tensor(
                            sl, pst[:], col_t[:, j0:j0 + 512], op=Alu.mult)
                    else:
                        ps = psum.tile([128, 512], f32, name="ps")
                        mm_group(ps, i0, j0, rb == 0 and jt == 0)
                        if cls == "upper":
                            mk = masks[(rb, jt)]
                            nc.vector.tensor_scalar(
                                mk[:], ps[:], thr_t[:], None, op0=Alu.is_gt)
                            nc.vector.tensor_tensor(
                                sl, mk[:], col_t[:, j0:j0 + 512], op=Alu.mult)
                        else:
                            nc.vector.scalar_tensor_tensor(
                                sl, ps[:], thr_t[:], col_t[:, j0:j0 + 512],
                                op0=Alu.is_gt, op1=Alu.mult)
                post(e1i[:], rb, 0, HALF, e0ip, outp)

                # jh=1: off-diagonal half, full compute
                last_block = (rb == RB - 1)
                if not last_block:
                    e1b = e1ip.tile([128, HALF], f32, name="e1i")
                    for jt in range(JT):
                        j0 = HALF + jt * 512
                        ps = psum.tile([128, 512], f32, name="ps")
                        mm_group(ps, i0, j0, False)
                        nc.vector.scalar_tensor_tensor(
                            e1b[:, jt * 512:(jt + 1) * 512], ps[:], thr_t[:],
                            col_t[:, j0:j0 + 512], op0=Alu.is_gt, op1=Alu.mult)
                    post(e1b[:], rb, HALF, HALF, e0ip, outp)
                else:
                    for jt in range(JT):
                        j0 = HALF + jt * 512
                        ps = psum.tile([128, 512], f32, name="ps")
                        mm_group(ps, i0, j0, False)
                        e1s = e1if.tile([128, 512], f32, name="e1s")
                        nc.vector.scalar_tensor_tensor(
                            e1s[:], ps[:], thr_t[:],
                            col_t[:, j0:j0 + 512], op0=Alu.is_gt, op1=Alu.mult)
                        post(e1s[:], rb, j0, 512, e0if, outf)
    nc.compile()
    return nc


def get_nc():
    global _compiled_nc
    if _compiled_nc is None:
        _compiled_nc = _build_nc()
    return _compiled_nc


def make_inputs(x):
    import ml_dtypes

    xs = np.asarray(x)[:, :, :, 0]                      # (B, C, N) fp32
    nrm = np.sqrt(np.sum(xs * xs, axis=1, keepdims=True))
    xn = xs / np.maximum(nrm, EPS)

    ident = np.eye(128, dtype=ml_dtypes.bfloat16)
    Nsq = float(N) * float(N)
    in_maps = []
    for b in range(B):
        xb64 = xn[b].astype(np.float64)
        s = xb64.sum(axis=1)
        M = xb64 @ xb64.T
        sum_g = float(s @ s)
        sum_g2 = float((M * M).sum())
        mean = (2.0 * sum_g - 2.0 * Nsq) / Nsq
        s2 = 4.0 * sum_g2 - 8.0 * sum_g + 4.0 * Nsq
        var = (s2 - Nsq * mean * mean) / (Nsq - 1.0)
        t_b = (mean + PPF_09 * np.sqrt(var) + 2.0) / 2.0
        thr_dev = np.full((128, 1), t_b * SCALE * SCALE, np.float32)

        xbs = (xn[b] * SCALE).astype(np.float32)
        for h in range(2):
            # local column order: own rows first, then the other half
            xloc = np.concatenate(
                [xbs[:, h * HALF:(h + 1) * HALF],
                 xbs[:, (1 - h) * HALF:(2 - h) * HALF]], axis=1)  # (C, N)
            hi = xloc.astype(np.float16)
            lo = (xloc - hi.astype(np.float32)).astype(np.float16)
            xhl = np.stack([hi.reshape(KT, 128, N),
                            lo.reshape(KT, 128, N)], axis=2)
            xh0 = np.ascontiguousarray(xhl[0, :, :, :HEADC])
            gcols = np.concatenate(
                [h * HALF + np.arange(HALF), (1 - h) * HALF + np.arange(HALF)])
            colp1 = np.ascontiguousarray(np.broadcast_to(
                (b * N + gcols + 1).astype(np.float32), (128, N)))
            rows = (b * N + h * HALF
                    + (np.arange(RB)[None, :] * 128 + np.arange(128)[:, None])
                    + 1).astype(np.float32)
            in_maps.append({
                "xh0": xh0,
                "xhl": np.ascontiguousarray(xhl),
                "thr": thr_dev,
                "rowp1": np.ascontiguousarray(rows),
                "colp1": colp1,
                "ident": ident,
            })
    return in_maps


def assemble(results):
    out = np.empty((2, B * N * N), np.int32)
    for c in range(NCORES):
        b, h = divmod(c, 2)
        base = b * N * N + h * HALF * N
        for plane, key in ((0, "e0"), (1, "e1")):
            dst = out[plane, base:base + HALF * N].reshape(HALF, 2, HALF)
            src = results[c][key].reshape(HALF, 2, HALF)
            if h == 0:
                dst[:] = src
            else:
                dst[:, 0, :] = src[:, 1, :]
                dst[:, 1, :] = src[:, 0, :]
    return out


def kernel(x):
    from concourse.bass_utils import run_bass_kernel_spmd

    nc = get_nc()
    in_maps = make_inputs(x)
    res = run_bass_kernel_spmd(nc, in_maps, list(range(NCORES)))
    return assemble(res.results)
